# revision 19
# baseline (speedup 1.0000x reference)
"""C3D-style circulant-block 3D CNN forward pass on 8 Trainium2 NeuronCores.

Sharding: data-parallel over batch (8 samples -> 8 cores). Training-mode
BatchNorm batch statistics are combined across cores with small per-chunk
f32 AllReduces of (mean, E[x^2]) per channel, pipelined so only the last
chunk's collective is exposed.

v2 layout highlights:
  conv1: z-plane pairs share one [128, 448] PSUM tile via tile_position
  column split; ACT copies PSUM->bf16; DVE does bn_stats + W/H pool maxes
  on 128 partitions; pooled output stays resident in SBUF as z-pairs.
  conv2: matmuls read the SBUF z-pair storage directly by parity
  (K=128 pair matmul + K=64 single at partition base 64/0), one ACT
  BN+ReLU build per plane-pair, border-z taps skipped.
  conv3a..5b: shift-and-accumulate implicit GEMM as before, with per-
  m-chunk stats AllReduce + apply pipelining.
"""

import numpy as np
import ml_dtypes

import concourse.bass as bass
import concourse.mybir as mybir
import concourse.tile as tile
from concourse import bacc
from concourse.bass_utils import run_bass_kernel_spmd

F32 = mybir.dt.float32
BF16 = mybir.dt.bfloat16
NPBF16 = ml_dtypes.bfloat16
RELU = mybir.ActivationFunctionType.Relu
COPY = mybir.ActivationFunctionType.Copy
SQRT = mybir.ActivationFunctionType.Sqrt
IDENT = mybir.ActivationFunctionType.Identity
SQUARE = getattr(mybir.ActivationFunctionType, "Square", None)
ADD = mybir.AluOpType.add
MULT = mybir.AluOpType.mult
EPS = 1e-5
N_CORES = 8

# name, Cin, Cout, D, H, W, R(rows/tile), zpair, pooled
GEN_LAYERS = [
    ("3a", 128, 256, 8, 28, 28, 14, False, False),
    ("3b", 256, 256, 8, 28, 28, 14, False, True),
    ("4a", 256, 512, 4, 14, 14, 14, True, False),
    ("4b", 512, 512, 4, 14, 14, 14, True, True),
    ("5a", 512, 512, 2, 7, 7, 7, True, False),
    ("5b", 512, 512, 2, 7, 7, 7, True, None),  # None -> raw stage (special pool)
]

TAPS = [(kd, kh, kw) for kd in range(3) for kh in range(3) for kw in range(3)]


def circ_expand_np(c):
    c = np.asarray(c, np.float32)
    P, Q, b = c.shape[0], c.shape[1], c.shape[2]
    r = np.arange(b)
    idx = (r[:, None] - r[None, :]) % b
    w = c[:, :, idx]  # (P, Q, b, b, k, k, k)
    w = np.transpose(w, (0, 2, 1, 3, 4, 5, 6))
    return w.reshape(P * b, Q * b, *c.shape[3:])


def pack_w_generic(wd, Kch, Mch):
    # wd (Co, Ci, 3,3,3) -> [Mch, 128ci, Kch, 27, 128co] bf16
    Co, Ci = wd.shape[0], wd.shape[1]
    wt = wd.transpose(1, 2, 3, 4, 0)  # (Ci, kd,kh,kw, Co)
    wt = wt.reshape(Kch, 128, 27, Mch, 128)
    wt = wt.transpose(3, 1, 0, 2, 4)  # (m, ci, c, t, co)
    return np.ascontiguousarray(wt, dtype=NPBF16)


def host_prep(inputs):
    g = {k: np.asarray(v, np.float32) for k, v in inputs.items()}
    shared = {}
    # conv1
    w1 = g["conv1_w"]  # (64, 3, 3,3,3)
    shared["w1"] = np.ascontiguousarray(
        w1.transpose(1, 2, 3, 4, 0).reshape(81, 64), dtype=NPBF16)
    # conv2 parity-paired weights
    w2 = circ_expand_np(g["c2"])  # (128, 64, 3,3,3)
    w2t = w2.transpose(2, 1, 3, 4, 0)  # (kd, ci, kh, kw, co)
    kd = [np.ascontiguousarray(w2t[i].reshape(64, 9, 128)) for i in range(3)]
    shared["w2A"] = np.concatenate([kd[1], kd[2]], axis=0).astype(NPBF16)  # even z pair
    shared["w2C"] = np.concatenate([kd[0], kd[1]], axis=0).astype(NPBF16)  # odd z pair
    shared["w2B"] = kd[0].astype(NPBF16)  # even z single (upper half)
    shared["w2D"] = kd[2].astype(NPBF16)  # odd z single (lower half)
    # generic layers
    for (name, Cin, Cout, *_rest) in GEN_LAYERS:
        wd = circ_expand_np(g[f"c{name}"])
        shared[f"w{name}"] = pack_w_generic(wd, Cin // 128, Cout // 128)
    # bn params
    def pk(v, parts):
        v = np.asarray(v, np.float32)
        mch = v.size // parts
        return np.ascontiguousarray(v.reshape(mch, parts).T)
    g1p = pk(g["g1"], 64)
    b1p = pk(g["b1"], 64)
    shared["gn1"] = np.ascontiguousarray(np.concatenate([g1p, g1p], axis=0))
    shared["bn1"] = np.ascontiguousarray(np.concatenate([b1p, b1p], axis=0))
    for name, c in [("2", 128), ("3a", 256), ("3b", 256), ("4a", 512),
                    ("4b", 512), ("5a", 512), ("5b", 512)]:
        shared[f"gn{name}"] = pk(g[f"g{name}"], 128)
        shared[f"bn{name}"] = pk(g[f"b{name}"], 128)
        assert np.all(g[f"g{name}"] >= 0), "pool/BN commute needs g >= 0"
    assert np.all(g["g1"] >= 0)
    # fc (fold /16 global-mean into weights)
    fcw = (g["fc_w"].T / 16.0)  # (512, 101)
    shared["fcw"] = np.ascontiguousarray(
        fcw.reshape(4, 128, 101).transpose(1, 0, 2), dtype=NPBF16)
    shared["fcb"] = np.ascontiguousarray(g["fc_b"].reshape(101, 1))
    # per-core conv1 im2col
    x = g["x"]  # (8, 3, 16, 112, 112)
    x1_list = []
    for i in range(x.shape[0]):
        xp = np.zeros((3, 18, 114, 114), np.float32)
        xp[:, 1:17, 1:113, 1:113] = x[i]
        sw = np.lib.stride_tricks.sliding_window_view(xp, (3, 3, 3), axis=(1, 2, 3))
        b1 = sw.transpose(0, 4, 5, 6, 1, 2, 3).reshape(81, 16, 12544)
        x1_list.append(np.ascontiguousarray(b1, dtype=NPBF16))
    return shared, x1_list


def build_bass(n_cores, fake_cc=False):
    nc = bacc.Bacc("TRN2", target_bir_lowering=False, debug=False,
                   num_devices=n_cores)
    rg = [list(range(n_cores))]

    din = {}
    din["x1"] = nc.dram_tensor("x1", [81, 16, 12544], BF16, kind="ExternalInput")
    din["w1"] = nc.dram_tensor("w1", [81, 64], BF16, kind="ExternalInput")
    for k in ("w2A", "w2C"):
        din[k] = nc.dram_tensor(k, [128, 9, 128], BF16, kind="ExternalInput")
    for k in ("w2B", "w2D"):
        din[k] = nc.dram_tensor(k, [64, 9, 128], BF16, kind="ExternalInput")
    for (name, Cin, Cout, *_r) in GEN_LAYERS:
        din[f"w{name}"] = nc.dram_tensor(
            f"w{name}", [Cout // 128, 128, Cin // 128, 27, 128], BF16,
            kind="ExternalInput")
    din["gn1"] = nc.dram_tensor("gn1", [128, 1], F32, kind="ExternalInput")
    din["bn1"] = nc.dram_tensor("bn1", [128, 1], F32, kind="ExternalInput")
    for name, c in [("2", 128), ("3a", 256), ("3b", 256), ("4a", 512),
                    ("4b", 512), ("5a", 512), ("5b", 512)]:
        mch = c // 128
        din[f"gn{name}"] = nc.dram_tensor(f"gn{name}", [128, mch], F32,
                                          kind="ExternalInput")
        din[f"bn{name}"] = nc.dram_tensor(f"bn{name}", [128, mch], F32,
                                          kind="ExternalInput")
    din["fcw"] = nc.dram_tensor("fcw", [128, 4, 101], BF16, kind="ExternalInput")
    din["fcb"] = nc.dram_tensor("fcb", [101, 1], F32, kind="ExternalInput")
    logits = nc.dram_tensor("logits", [101, 1], F32, kind="ExternalOutput")

    with tile.TileContext(nc) as tc:
        build_graph(tc, din, logits, rg, fake_cc)
    nc.compile()
    return nc


def build_graph(tc, din, logits, rg, fake_cc=False):
    nc = tc.nc
    import contextlib
    ctx = contextlib.ExitStack()
    with ctx:
        singles = ctx.enter_context(tc.tile_pool(name="singles", bufs=1))
        bnp = ctx.enter_context(tc.tile_pool(name="bnp", bufs=1))
        statsp = ctx.enter_context(tc.tile_pool(name="statsp", bufs=2))
        ybfp = ctx.enter_context(tc.tile_pool(name="ybfp", bufs=2))
        pwp = ctx.enter_context(tc.tile_pool(name="pwp", bufs=2))
        ypoolp = ctx.enter_context(tc.tile_pool(name="ypoolp", bufs=1))
        dram = ctx.enter_context(tc.tile_pool(name="dram", bufs=1, space="DRAM"))

        eps_t = singles.tile([128, 1], F32, tag="eps")
        nc.gpsimd.memset(eps_t[:], EPS)

        # persistent small params
        params = {}
        for name in ("1", "2", "3a", "3b", "4a", "4b", "5a", "5b"):
            mch = din[f"gn{name}"].shape[1]
            gt = singles.tile([128, mch], F32, tag=f"g{name}")
            bt = singles.tile([128, mch], F32, tag=f"b{name}")
            nc.sync.dma_start(gt[:], din[f"gn{name}"][:])
            nc.sync.dma_start(bt[:], din[f"bn{name}"][:])
            params[name] = (gt, bt)

        w1_sb = singles.tile([81, 64], BF16, tag="w1")
        nc.sync.dma_start(w1_sb[:], din["w1"][:])
        w2A_sb = singles.tile([128, 9, 128], BF16, tag="w2A")
        nc.sync.dma_start(w2A_sb[:], din["w2A"][:])
        w2C_sb = singles.tile([128, 9, 128], BF16, tag="w2C")
        nc.sync.dma_start(w2C_sb[:], din["w2C"][:])
        w2B_sb = singles.tile([128, 9, 128], BF16, tag="w2B")
        nc.sync.dma_start(w2B_sb[64:128], din["w2B"][:])
        w2D_sb = singles.tile([128, 9, 128], BF16, tag="w2D")
        nc.sync.dma_start(w2D_sb[0:64], din["w2D"][:])
        fcw_sb = singles.tile([128, 4, 101], BF16, tag="fcw")
        nc.sync.dma_start(fcw_sb[:], din["fcw"][:])
        fcb_sb = singles.tile([101, 1], F32, tag="fcb")
        nc.sync.dma_start(fcb_sb[:], din["fcb"][:])

        def collective(tag, cc_ap, parts, w2):
            """AllReduce a [parts, w2] f32 payload; returns SBUF tile."""
            ccin = dram.tile([parts, w2], F32, tag=f"ci{tag}")
            ccout = dram.tile([parts, w2], F32, tag=f"co{tag}",
                              addr_space="Shared")
            nc.sync.dma_start(ccin[:], cc_ap)
            if fake_cc:
                nc.sync.dma_start(ccout[:], ccin[:])
            else:
                nc.gpsimd.collective_compute(
                    "AllReduce", ADD, replica_groups=rg,
                    ins=[ccin.opt()], outs=[ccout.opt()])
            ar = bnp.tile([parts, w2], F32, tag=f"ar{tag}")
            nc.sync.dma_start(ar[:], ccout[:])
            return ar

        def bn_post(tag, ar_sum, parts, W, inv_n, g_ap, b_ap, s_dst, t_dst):
            """ar_sum [parts, W, 2] summed stats -> s,t written to dst slices.

            Engine spread keeps <=4 waiting ops per engine queue."""
            mge = bnp.tile([parts, W, 2], F32, tag=f"mge{tag}")
            nc.gpsimd.tensor_scalar(mge[:], ar_sum, inv_n, None, op0=MULT)
            sq = bnp.tile([parts, W], F32, tag=f"sq{tag}")
            if SQUARE is not None:
                nc.scalar.activation(sq[:], mge[:, :, 0], SQUARE)
            else:
                nc.vector.tensor_mul(sq[:], mge[:, :, 0], mge[:, :, 0])
            varg = bnp.tile([parts, W], F32, tag=f"vg{tag}")
            nc.vector.tensor_sub(varg[:], mge[:, :, 1], sq[:])
            sd = bnp.tile([parts, W], F32, tag=f"sd{tag}")
            nc.scalar.activation(sd[:], varg[:], SQRT, bias=eps_t[:parts])
            inv = bnp.tile([parts, W], F32, tag=f"inv{tag}")
            nc.vector.reciprocal(inv[:], sd[:])
            nc.vector.tensor_mul(s_dst, inv[:], g_ap)
            tmn = bnp.tile([parts, W], F32, tag=f"tm{tag}")
            nc.vector.tensor_mul(tmn[:], mge[:, :, 0], s_dst)
            nc.vector.tensor_sub(t_dst, b_ap, tmn[:])

        # ---------------- conv1 ----------------
        # y1sb: pooled conv1 output, z-pairs on partition halves, RAW (pre-BN)
        y1ctx = tc.tile_pool(name="y1p", bufs=1)
        y1pool = y1ctx.__enter__()
        y1sb = y1pool.tile([128, 8, 3136], BF16, tag="y1sb")
        y1v = y1sb[:].rearrange("p z (h w) -> p z h w", h=56)
        stats1 = statsp.tile([128, 224, 6], F32, tag="stats")
        part1 = bnp.tile([128, 8, 2], F32, tag="part1")
        s1d = bnp.tile([128, 1], F32, tag="s1d")
        t1d = bnp.tile([128, 1], F32, tag="t1d")
        ar1 = [None, None]
        with tc.tile_pool(name="x1p", bufs=3) as x1p, \
             tc.tile_pool(name="ycpp", bufs=3) as ycpp, \
             tc.tile_pool(name="ps1", bufs=4, space="PSUM") as ps1, \
             nc.named_scope("conv1"):
            for zp in range(8):
                for half in range(2):
                    slab = x1p.tile([81, 2, 6272], BF16, tag="slab")
                    nc.sync.dma_start(
                        slab[:],
                        din["x1"][:, 2 * zp:2 * zp + 2,
                                  half * 6272:(half + 1) * 6272])
                    for t in range(7):
                        ycp = ycpp.tile([128, 2, 448], BF16, tag="ycp",
                                        name="ycp")
                        for u in range(2):
                            pst = ps1.tile([128, 448], F32, tag=f"ps{u}",
                                           name="ps")
                            c0 = (2 * t + u) * 448
                            nc.tensor.matmul(pst[0:64, :], w1_sb[:],
                                             slab[:, 0, c0:c0 + 448],
                                             start=True, stop=True,
                                             tile_position=(0, 0))
                            nc.tensor.matmul(pst[64:128, :], w1_sb[:],
                                             slab[:, 1, c0:c0 + 448],
                                             start=True, stop=True,
                                             tile_position=(0, 64))
                            nc.scalar.activation(ycp[:, u], pst[:], COPY)
                            ti = zp * 28 + half * 14 + 2 * t + u
                            nc.vector.bn_stats(stats1[:, ti], pst[:])
                        # W-pool then H-pool (8 plane rows -> 4 pooled rows)
                        v = ycp[:].rearrange("p u (a b) -> p (u a) b", a=4)
                        pw = pwp.tile([128, 8, 56], BF16, tag="pw")
                        nc.vector.tensor_max(pw[:], v[:, :, 0::2],
                                             v[:, :, 1::2])
                        r0 = half * 28 + 4 * t
                        nc.vector.tensor_max(y1v[:, zp, r0:r0 + 4, :],
                                             pw[:, 0::2, :], pw[:, 1::2, :])
                # partial stats per z-pair (28 tiles, equal counts)
                nc.vector.bn_aggr(part1[:, zp], stats1[:, zp * 28:zp * 28 + 28])
                if zp == 3 or zp == 7:
                    h = zp // 4
                    with nc.named_scope(f"ar1{h}"):
                        e2p = bnp.tile([128, 4, 2], F32, tag=f"e2p{h}")
                        ph = part1[:, 4 * h:4 * h + 4]
                        if SQUARE is not None:
                            nc.scalar.activation(e2p[:, :, 0], ph[:, :, 0],
                                                 SQUARE)
                        else:
                            nc.vector.tensor_mul(e2p[:, :, 0], ph[:, :, 0],
                                                 ph[:, :, 0])
                        nc.vector.tensor_add(e2p[:, :, 1], ph[:, :, 1],
                                             e2p[:, :, 0])
                        st = bnp.tile([128, 2], F32, tag=f"st1{h}")
                        nc.vector.tensor_reduce(
                            st[:, 0:1], ph[:, :, 0:1],
                            axis=mybir.AxisListType.XY, op=ADD)
                        nc.vector.tensor_reduce(
                            st[:, 1:2], e2p[:, :, 1:2],
                            axis=mybir.AxisListType.XY, op=ADD)
                        # fold z-odd (upper 64) into z-even (lower 64)
                        fold = bnp.tile([64, 2], F32, tag=f"fold{h}")
                        nc.sync.dma_start(fold[:], st[64:128, :])
                        cc = bnp.tile([64, 2], F32, tag=f"cc1{h}")
                        nc.vector.tensor_add(cc[:], st[0:64, :], fold[:])
                        ar1[h] = collective(f"c1{h}", cc[:], 64, 2)
        with nc.named_scope("ar1"):
            # broadcast both AR results to 128 partitions, sum, then post
            a0 = bnp.tile([128, 2], F32, tag="ar1b0")
            a1 = bnp.tile([128, 2], F32, tag="ar1b1")
            for h, dstt in ((0, a0), (1, a1)):
                nc.sync.dma_start(dstt[0:64, :], ar1[h][:])
                nc.sync.dma_start(dstt[64:128, :], ar1[h][:])
            ars = bnp.tile([128, 2], F32, tag="ars1")
            nc.vector.tensor_add(ars[:], a0[:], a1[:])
            gt, bt = params["1"]
            bn_post("c1", ars[:].rearrange("p (w two) -> p w two", w=1),
                    128, 1, 1.0 / 128, gt[:], bt[:], s1d[:], t1d[:])

        # ---------------- conv2 ----------------
        stats2 = statsp.tile([128, 2, 64, 6], F32, tag="stats")
        Y2p = ypoolp.tile([128, 8, 28, 28], BF16, tag="ypool2")
        s2t = bnp.tile([128, 1], F32, tag="s2t")
        t2t = bnp.tile([128, 1], F32, tag="t2t")
        ar2 = [None, None]
        p2tiles = {}

        with tc.tile_pool(name="p2p", bufs=4) as p2p, \
             tc.tile_pool(name="ps2", bufs=3, space="PSUM") as ps2:

            def build_p2(zp):
                P = p2p.tile([128, 58, 58], BF16, tag="p2")
                nc.gpsimd.memset(P[:, 0, :], 0.0)
                nc.gpsimd.memset(P[:, 57, :], 0.0)
                nc.gpsimd.memset(P[:, 1:57, 0:1], 0.0)
                nc.gpsimd.memset(P[:, 1:57, 57:58], 0.0)
                nc.scalar.activation(P[:, 1:57, 1:57], y1v[:, zp], RELU,
                                     bias=t1d[:, 0:1], scale=s1d[:, 0:1])
                p2tiles[zp] = P

            def conv2_z(z):
                zp = z // 2
                if z % 2 == 0:
                    pairT, wP = p2tiles[zp], w2A_sb
                    single = (p2tiles[zp - 1], 64, w2B_sb) if z > 0 else None
                else:
                    pairT, wP = p2tiles[zp], w2C_sb
                    single = (p2tiles[zp + 1], 0, w2D_sb) if z < 15 else None
                for p2 in range(4):
                    pst = ps2.tile([128, 2, 512], F32, tag="ps", name="ps")
                    for k9 in range(9):
                        kh, kw = k9 // 3, k9 % 3
                        for j in range(2):
                            y0 = 14 * p2 + 7 * j + kh
                            nc.tensor.matmul(
                                pst[:, j, :392], wP[:, k9],
                                pairT[:, y0:y0 + 7, kw:kw + 56],
                                start=(k9 == 0),
                                stop=(single is None and k9 == 8))
                    if single:
                        sT, pb, wS = single
                        for k9 in range(9):
                            kh, kw = k9 // 3, k9 % 3
                            for j in range(2):
                                y0 = 14 * p2 + 7 * j + kh
                                nc.tensor.matmul(
                                    pst[:, j, :392], wS[pb:pb + 64, k9],
                                    sT[pb:pb + 64, y0:y0 + 7, kw:kw + 56],
                                    start=False, stop=(k9 == 8),
                                    tile_position=(pb, 0))
                    ybft = ybfp.tile([128, 1024], BF16, tag="ybf", name="ybf")
                    ybf = ybft[:, :784]
                    nc.scalar.activation(
                        ybf.rearrange("p (g n) -> p g n", g=2),
                        pst[:, :, :392], COPY)
                    h2 = z // 8
                    ti = (z % 8) * 8 + p2 * 2
                    nc.vector.bn_stats(stats2[:, h2, ti], ybft[:, 0:392])
                    nc.vector.bn_stats(stats2[:, h2, ti + 1], ybft[:, 392:784])
                    v = ybf.rearrange("p (a b) -> p a b", a=14)
                    pw = pwp.tile([128, 14, 28], BF16, tag="pw2")
                    nc.vector.tensor_max(pw[:], v[:, :, 0::2], v[:, :, 1::2])
                    dst = Y2p[:, z // 2, 7 * p2:7 * p2 + 7, :]
                    if z % 2 == 0:
                        nc.vector.tensor_max(dst, pw[:, 0::2, :],
                                             pw[:, 1::2, :])
                    else:
                        hw = pwp.tile([128, 7, 28], BF16, tag="pwh")
                        nc.vector.tensor_max(hw[:], pw[:, 0::2, :],
                                             pw[:, 1::2, :])
                        nc.vector.tensor_max(dst, dst, hw[:])

            def conv2_ar(h):
                with nc.named_scope(f"ar2{h}"):
                    mv = bnp.tile([128, 2], F32, tag=f"mv2{h}")
                    nc.vector.bn_aggr(mv[:], stats2[:, h])
                    cc = bnp.tile([128, 2], F32, tag=f"cc2{h}")
                    if SQUARE is not None:
                        nc.scalar.activation(cc[:, 0:1], mv[:, 0:1], SQUARE)
                    else:
                        nc.vector.tensor_mul(cc[:, 0:1], mv[:, 0:1], mv[:, 0:1])
                    nc.vector.tensor_add(cc[:, 1:2], mv[:, 1:2], cc[:, 0:1])
                    nc.vector.tensor_copy(cc[:, 0:1], mv[:, 0:1])
                    ar2[h] = collective(f"c2{h}", cc[:], 128, 2)

            with nc.named_scope("conv2"):
                build_p2(0)
                build_p2(1)
                for zp in range(1, 8):
                    conv2_z(2 * zp - 2)
                    conv2_z(2 * zp - 1)
                    if zp == 4:
                        conv2_ar(0)
                    build_p2(zp + 1) if zp < 7 else None
                conv2_z(14)
                conv2_z(15)
                conv2_ar(1)
        y1ctx.__exit__(None, None, None)
        stagep = ctx.enter_context(tc.tile_pool(name="stagep", bufs=1))
        arena = ctx.enter_context(tc.tile_pool(name="arena", bufs=1))
        with nc.named_scope("ar2"):
            arsum2 = bnp.tile([128, 2], F32, tag="ars2")
            nc.vector.tensor_add(arsum2[:], ar2[0][:], ar2[1][:])
            gt, bt = params["2"]
            bn_post("c2", arsum2[:].rearrange("p (w two) -> p w two", w=1),
                    128, 1, 1.0 / 16, gt[:], bt[:], s2t[:], t2t[:])
        P3in = arena.tile([128, 1, 10, 30, 30], BF16, tag="pin3a")
        zero_borders(nc, P3in, 1, 10, 30, 30)
        for h in range(2):
            nc.scalar.activation(P3in[:, 0, 1 + 4 * h:5 + 4 * h, 1:29, 1:29],
                                 Y2p[:, 4 * h:4 * h + 4], RELU,
                                 bias=t2t[:, 0:1], scale=s2t[:, 0:1])

        # ---------------- generic conv layers ----------------
        with tc.tile_pool(name="wp", bufs=2) as wp, \
             tc.tile_pool(name="ps3", bufs=3, space="PSUM") as ps3, \
             tc.tile_pool(name="psfc", bufs=1, space="PSUM") as psfc:
            Pin = P3in
            for (name, Cin, Cout, D, H, W, R, zpair, pooled) in GEN_LAYERS:
                Kch, Mch = Cin // 128, Cout // 128
                ntz = D // 2 if zpair else D
                zcnt = 2 if zpair else 1
                ytiles = H // R
                N = zcnt * R * W
                T = ntz * ytiles
                H2, W2, D2 = H // 2, W // 2, D // 2
                gt, bt = params[name]
                stats_t = statsp.tile([128, Mch, T, 6], F32, tag="stats")
                s_all = bnp.tile([128, Mch], F32, tag=f"sa{name}")
                t_all = bnp.tile([128, Mch], F32, tag=f"ta{name}")
                if pooled is False or pooled is None:
                    stage = stagep.tile([128, Mch, D, H, W], BF16, tag="stage")
                else:
                    stage = stagep.tile([128, Mch, D, H2, W2], BF16, tag="stage")
                stage_flat = stage[:].rearrange("p m d h w -> p (m d h w)")
                if pooled:
                    ysrc = ypoolp.tile([128, Mch, D2, H2, W2], BF16,
                                       tag="ypool")
                tiles = [(2 * tz if zpair else tz, ty * R)
                         for tz in range(ntz) for ty in range(ytiles)]
                groups = [tiles[i:i + 2] for i in range(0, len(tiles), 2)]
                # next-layer padded input (skip for 5b: special tail)
                if name != "5b":
                    if pooled:
                        nD, nH, nW = D2, H2, W2
                    else:
                        nD, nH, nW = D, H, W
                    nKch = Mch
                    Pnext = arena.tile([128, nKch, nD + 2, nH + 2, nW + 2],
                                       BF16, tag=f"pin{name}")
                    zero_borders(nc, Pnext, nKch, nD + 2, nH + 2, nW + 2)

                gw = {"3a": 1, "3b": 1, "4a": 2, "4b": 2,
                      "5a": 4, "5b": 4}[name]
                ccg = None
                scope = nc.named_scope(f"conv{name}")
                scope.__enter__()
                for m in range(Mch):
                    wm = wp.tile([128, Kch, 27, 128], BF16, tag="w")
                    nc.sync.dma_start(wm[:, :Kch], din[f"w{name}"][m])
                    ti = 0
                    for grp in groups:
                        G = len(grp)
                        pst = ps3.tile([128, 2, 512], F32, tag="ps", name="ps")
                        nmm = Kch * 27
                        i = 0
                        for c in range(Kch):
                            for (kd, kh, kw) in TAPS:
                                for j, (z0, y0) in enumerate(grp):
                                    rhs = Pin[:, c, z0 + kd:z0 + kd + zcnt,
                                              y0 + kh:y0 + kh + R,
                                              kw:kw + W]
                                    nc.tensor.matmul(
                                        pst[:, j, :N],
                                        wm[:, c, kd * 9 + kh * 3 + kw, :],
                                        rhs, start=(i == 0),
                                        stop=(i == nmm - 1))
                                i += 1
                        z0, y0 = grp[0]
                        if pooled is False or pooled is None:
                            off = (m * D + z0) * H * W + y0 * W
                            dst = stage_flat[:, off:off + G * N]
                            nc.vector.tensor_copy(
                                dst.rearrange("p (g n) -> p g n", g=G),
                                pst[:, :G, :N])
                            for j in range(G):
                                nc.vector.bn_stats(
                                    stats_t[:, m, ti + j],
                                    stage_flat[:, off + j * N:off + (j + 1) * N])
                        else:
                            ybft = ybfp.tile([128, 1024], BF16,
                                             tag="ybf", name="ybf")
                            ybf = ybft[:, :G * N]
                            nc.vector.tensor_copy(
                                ybf.rearrange("p (g n) -> p g n", g=G),
                                pst[:, :G, :N])
                            for j in range(G):
                                nc.vector.bn_stats(
                                    stats_t[:, m, ti + j],
                                    ybft[:, j * N:(j + 1) * N])
                            nz = G * zcnt if zpair else 1
                            nr = R if zpair else G * R
                            v = ybf.rearrange("p (z y x) -> p z y x",
                                              z=nz, y=nr)
                            pw = pwp.tile([128, nz, nr, W2], BF16,
                                          tag="pw3", name="pw")
                            nc.vector.tensor_max(pw[:], v[:, :, :, 0::2],
                                                 v[:, :, :, 1::2])
                            nc.vector.tensor_max(
                                stage[:, m, z0:z0 + nz,
                                      y0 // 2:y0 // 2 + nr // 2, :],
                                pw[:, :, 0::2, :], pw[:, :, 1::2, :])
                        ti += G
                    # m-chunk done: D-pool, then stats AR (pipelined)
                    if pooled:
                        nc.vector.tensor_max(ysrc[:, m], stage[:, m, 0::2],
                                             stage[:, m, 1::2])
                    gi = m % gw
                    if gi == 0:
                        ccg = bnp.tile([128, gw, 2], F32, tag=f"cc{name}{m}")
                    with nc.named_scope(f"ar{name}{m}"):
                        mv = bnp.tile([128, 2], F32, tag=f"mv{name}{m}")
                        nc.vector.bn_aggr(mv[:], stats_t[:, m])
                        if SQUARE is not None:
                            nc.scalar.activation(ccg[:, gi, 0:1], mv[:, 0:1],
                                                 SQUARE)
                        else:
                            nc.vector.tensor_mul(ccg[:, gi, 0:1], mv[:, 0:1],
                                                 mv[:, 0:1])
                        nc.vector.tensor_add(ccg[:, gi, 1:2], mv[:, 1:2],
                                             ccg[:, gi, 0:1])
                        nc.vector.tensor_copy(ccg[:, gi, 0:1], mv[:, 0:1])
                        if gi == gw - 1:
                            m0 = m - gw + 1
                            ar = collective(
                                f"{name}{m0}",
                                ccg[:].rearrange("p w two -> p (w two)"),
                                128, 2 * gw)
                            bn_post(f"{name}{m0}",
                                    ar[:].rearrange("p (w two) -> p w two",
                                                    w=gw),
                                    128, gw, 1.0 / 8, gt[:, m0:m0 + gw],
                                    bt[:, m0:m0 + gw], s_all[:, m0:m0 + gw],
                                    t_all[:, m0:m0 + gw])
                            if name != "5b":
                                src = ysrc if pooled else stage
                                for mm in range(m0, m + 1):
                                    nc.scalar.activation(
                                        Pnext[:, mm, 1:1 + nD, 1:1 + nH,
                                              1:1 + nW],
                                        src[:, mm], RELU,
                                        bias=t_all[:, mm:mm + 1],
                                        scale=s_all[:, mm:mm + 1])
                scope.__exit__(None, None, None)

                if name == "5b":
                    # pool5: window (2,2,2) stride 2, pad (0,1,1); stage is
                    # raw [128, 4, 2, 7, 7]
                    with nc.named_scope("tail"):
                        pd = bnp.tile([128, 4, 7, 7], BF16, tag="pd5")
                        nc.vector.tensor_max(pd[:], stage[:, :, 0],
                                             stage[:, :, 1])
                        pw5 = bnp.tile([128, 4, 7, 4], BF16, tag="pw5")
                        nc.vector.tensor_copy(pw5[:, :, :, 0:1],
                                              pd[:, :, :, 0:1])
                        nc.vector.tensor_max(pw5[:, :, :, 1:4],
                                             pd[:, :, :, 1::2],
                                             pd[:, :, :, 2::2])
                        ph5 = bnp.tile([128, 4, 4, 4], BF16, tag="ph5")
                        nc.vector.tensor_copy(ph5[:, :, 0:1, :],
                                              pw5[:, :, 0:1, :])
                        nc.vector.tensor_max(ph5[:, :, 1:4, :],
                                             pw5[:, :, 1::2, :],
                                             pw5[:, :, 2::2, :])
                        Z = bnp.tile([128, 4, 16], BF16, tag="z5")
                        for m in range(4):
                            nc.scalar.activation(
                                Z[:, m, :],
                                ph5[:, m].rearrange("p a b -> p (a b)"),
                                RELU, bias=t_all[:, m:m + 1],
                                scale=s_all[:, m:m + 1])
                        feat = bnp.tile([128, 4], F32, tag="feat")
                        nc.vector.tensor_reduce(feat[:], Z[:],
                                                axis=mybir.AxisListType.X,
                                                op=ADD)
                        fcin = bnp.tile([128, 4], BF16, tag="fcin")
                        nc.vector.tensor_copy(fcin[:], feat[:])
                        psf = psfc.tile([101, 1], F32, tag="psfc", name="psfc")
                        for c in range(4):
                            nc.tensor.matmul(psf[:], fcw_sb[:, c, :],
                                             fcin[:, c:c + 1],
                                             start=(c == 0), stop=(c == 3))
                        out_sb = bnp.tile([101, 1], F32, tag="outsb")
                        nc.scalar.activation(out_sb[:], psf[:], IDENT,
                                             bias=fcb_sb[:])
                        nc.sync.dma_start(logits[:], out_sb[:])
                    break
                Pin = Pnext


def zero_borders(nc, P, mch, Dp, Hp, Wp):
    for c in range(mch):
        nc.gpsimd.memset(P[:, c, 0], 0.0)
        nc.gpsimd.memset(P[:, c, Dp - 1], 0.0)
        nc.gpsimd.memset(P[:, c, 1:Dp - 1, 0, :], 0.0)
        nc.gpsimd.memset(P[:, c, 1:Dp - 1, Hp - 1, :], 0.0)
        nc.gpsimd.memset(P[:, c, 1:Dp - 1, 1:Hp - 1, 0:1], 0.0)
        nc.gpsimd.memset(P[:, c, 1:Dp - 1, 1:Hp - 1, Wp - 1:Wp], 0.0)


_STATE = {}


def _get_nc(n_cores=N_CORES):
    key = f"nc{n_cores}"
    if key not in _STATE:
        _STATE[key] = build_bass(n_cores)
    return _STATE[key]


def kernel(**inputs):
    nc = _get_nc()
    shared, x1_list = host_prep(inputs)
    in_maps = []
    for i in range(N_CORES):
        m = dict(shared)
        m["x1"] = x1_list[i]
        in_maps.append(m)
    res = run_bass_kernel_spmd(nc, in_maps, core_ids=list(range(N_CORES)))
    out = np.stack([res.results[i]["logits"].reshape(101)
                    for i in range(N_CORES)]).astype(np.float32)
    return out
